# revision 71
# baseline (speedup 1.0000x reference)
"""Distributed multi-head attention kernel for 8 TRN2 NeuronCores.

Sharding: core c handles batch b = c//2 and head-group hg = c%2 (4 of 8
heads = 256 output columns).  Output slices are disjoint -> no collectives;
the host concatenates the 8 slices (bf16 device output, f32 host).

Device algorithm (per core), bf16 matmuls / f32 softmax:
  - host compacts BOTH axes: keys permuted unmasked-first (sparse
    attention over v_mask) and queries compacted to q_mask==1 only
    (dead queries are exactly zero in the reference); only NU=
    ceil(max_unmasked/128) key chunks and NQ=ceil(max_live_q/512)
    query tiles enter the pipeline.  Causal-mask thresholds move to
    compacted index space via host-side searchsorted, so the device
    masking (iota + per-partition is_ge) is unchanged.
  - DMA is descriptor-rate bound (~61ns per <=1KB row on each of 16
    SDMA engines), so inputs are packed per 128-partition quadrant
    into one [128, W] dram tensor with ~4KB rows: a "tranche" segment
    (wall | k 0:512 | v 0:128 | q 0:512) that pass (0,0) consumes,
    split [wall|k]/[v|q] so the k projection starts early, then bulk
    segments in consumption order (v | k | q) -- finer splits lose:
    each DMA issue costs ~690ns of sequencer time.  The scalar ring
    carries only tranche quadrants 0-1 (its compute instructions
    inherit waits on DMAs issued from the same queue, and the first
    exp must not wait on bulk); sync carries the rest; side data +
    iota ride gpsimd-SWDGE (small only: tiny-row descriptors on the
    sync ring would delay the tranche).  Output is written per pass
    as a contiguous [128, 512] block (1KB rows) of the persistent
    ofin tile; the host re-interleaves blocks b=2t+dc.
  - scores in S^T layout [k', q]; the two heads of a pair use PE row
    groups 0-63 / 64-127 so their score matmuls run concurrently; one
    [128, 1024] PSUM tile holds both heads' scores for a q-tile and a
    single ACT exp (per-partition key bias; scale=0.125) covers both
  - causal masking: block-level skips from a union-over-batches
    liveness structure (SPMD-identical graph); straddling blocks are
    trimmed to q >= qlo (union) in scores/exp/mask/PV, and the
    residual staircase masks are generated on device (gpsimd iota +
    vector is_ge against per-band thresholds)
  - PV: O^T[65, q] accumulated in PSUM over key chunks; row 64 (ones
    column appended to VW) is the softmax denominator
  - the (q-tile t, head-pair dc) passes are emitted t-major; all
    projections go into a global deadline queue drained one-or-more
    items per chunk across every pass, bounding projection bursts at
    tile boundaries; scores are emitted one chunk ahead of the
    previous chunk's PV (psS bufs=2 is scores-only, proj psum has its
    own 2-buf pool) so the in-order PE never waits on the
    exp -> mask -> PV chain; ALL psum->SBUF copies ride DVE because
    anything queued ahead of an exp on the scalar queue delays the
    exp stream; the psO->SBUF copies run at pass end (psO bufs=2,
    hidden under the next pass's first exp) while transposes + output
    scaling are deferred to the next pass's third chunk
  - dead queries (all causally-allowed keys masked but q_mask==1):
    host precomputes fvec = v_perm^T @ F at their compacted indices
    (they compact to the first columns); 16 tiny matmuls add the fix
    into output columns 0..3, with dead-slot counts joined at finalize
  - finalize per pass: DVE-copy psO to SBUF (bf16), PE-transpose to
    [q, 65], scale by q_mask/rowsum into ofin; host scatters the
    compacted rows back to full [S, 512]
"""

import numpy as np
import ml_dtypes

BF = ml_dtypes.bfloat16
B, S, D = 4, 2048, 512
HG = 256          # output columns per core (4 heads x 64)
KS = 65           # head value width + ones column
NCH = 16          # total key chunks of 128
NEG = np.float32(-1e10)

_CACHE = {}


def _structure(v_mask, q_mask):
    """Key/query compaction + block liveness (union over batches)."""
    perms, n1s, qposs = [], [], []
    for b in range(B):
        unm = np.where(v_mask[b] == 1)[0]
        msk = np.where(v_mask[b] == 0)[0]
        perms.append(np.concatenate([unm, msk]))
        n1s.append(len(unm))
        qposs.append(np.where(q_mask[b] == 1)[0])
    NU = int(max(-(-n // 128) for n in n1s))
    NQ = int(max(-(-len(qp) // 512) for qp in qposs))
    live = set()
    band = set()
    qlo_raw = {}
    for b in range(B):
        unm = perms[b][:n1s[b]]
        qp = qposs[b]
        nq = len(qp)
        for c in range(NU):
            seg = unm[128 * c:min(128 * (c + 1), n1s[b])]
            if len(seg) == 0:
                continue
            lo, hi = int(seg[0]), int(seg[-1])
            # compacted index of the first query that sees lo / all of hi
            qlo_c = int(np.searchsorted(qp, lo))
            qhi_c = int(np.searchsorted(qp, hi))
            for t in range(NQ):
                tile_last = min(512 * (t + 1), nq) - 1
                if tile_last < 512 * t or qlo_c > tile_last:
                    continue
                live.add((c, t))
                ql = max(0, qlo_c - 512 * t)
                qlo_raw[(c, t)] = min(qlo_raw.get((c, t), 512), ql)
                if qhi_c > 512 * t:
                    band.add((c, t))
    live_lists = tuple(tuple(sorted(c for (c, tt) in live if tt == t))
                       for t in range(NQ))
    band_list = tuple(sorted(band))
    # queries below qlo see no key of the chunk (union over batches); the
    # first live chunk of each tile keeps full width (starts the psO group)
    qlo = {}
    for (c, t), v in qlo_raw.items():
        qlo[(c, t)] = 0 if c == live_lists[t][0] else (v // 8) * 8
    qlo_t = tuple(sorted(qlo.items()))
    return perms, n1s, qposs, NU, NQ, live_lists, band_list, qlo_t


def _offsets(NU, NQ):
    """Column offsets inside the packed per-quadrant input tensor."""
    klim, NQT = NU * 128, NQ * 512
    o = {}
    o["wall"] = 0
    o["kta"] = 768
    o["vta"] = 768 + 512
    o["qta"] = o["vta"] + 128
    o["tr_end"] = o["qta"] + 512          # tranche = first-pass feed
    o["vtb"] = o["tr_end"]
    o["ktb"] = o["vtb"] + (klim - 128)
    o["qtb"] = o["ktb"] + (klim - 512)
    o["end"] = o["qtb"] + (NQT - 512)
    return o


def _build(NU, NQ, live_lists, band_list, qlo_t):
    import concourse.bass as bass  # noqa: F401
    from concourse import bacc
    import concourse.mybir as mybir
    from concourse.tile import TileContext

    F32 = mybir.dt.float32
    BF16 = mybir.dt.bfloat16
    I32 = mybir.dt.int32
    Exp = mybir.ActivationFunctionType.Exp
    nband = len(band_list)
    band_idx = {ct: i for i, ct in enumerate(band_list)}
    qlo = dict(qlo_t)
    klim = NU * 128
    NQT = NQ * 512
    kp_tiles = -(-klim // 512)  # s-tiles of K to project
    OF = _offsets(NU, NQ)

    nc = bacc.Bacc()
    xin = nc.declare_dram_parameter("xin", [D, OF["end"]], BF16, isOutput=False)
    vbias = nc.declare_dram_parameter("vbias", [128, NCH], F32, isOutput=False)
    qmask = nc.declare_dram_parameter("qmask", [128, 4 * NQ], F32, isOutput=False)
    bthr = nc.declare_dram_parameter("bthr", [128, nband], F32, isOutput=False)
    fvec = nc.declare_dram_parameter("fvec", [128, 16], BF16, isOutput=False)
    cnt = nc.declare_dram_parameter("cnt", [128, 4], F32, isOutput=False)
    ident = nc.declare_dram_parameter("ident", [128, 128], BF16, isOutput=False)
    out = nc.declare_dram_parameter("out", [128, 4 * NQ * HG], BF16, isOutput=True)

    with TileContext(nc) as tc:
        with tc.tile_pool(name="sb", bufs=1) as sb, \
             tc.tile_pool(name="ps", bufs=1, space="PSUM") as ps:

            def sbt(name, shape, dtype, bufs=1, tag=None):
                return sb.tile(shape, dtype, name=name, tag=tag or name, bufs=bufs)

            xs = [sb.tile([128, OF["end"]], BF16, name=f"xin{Dc}",
                          tag=f"xin{Dc}", bufs=1) for Dc in range(4)]

            w_sb = {}
            for Dc in range(4):
                for j, nm in enumerate(("q", "k", "v")):
                    w_sb[(nm, Dc)] = xs[Dc][:, HG * j:HG * (j + 1)]

            def kt_view(Dc, c0, c1):
                if c1 <= 512:
                    return xs[Dc][:, OF["kta"] + c0:OF["kta"] + c1]
                return xs[Dc][:, OF["ktb"] + c0 - 512:OF["ktb"] + c1 - 512]

            def vt_view(Dc, st):
                if st == 0:
                    return xs[Dc][:, OF["vta"]:OF["vta"] + 128]
                return xs[Dc][:, OF["vtb"] + 128 * (st - 1):OF["vtb"] + 128 * st]

            def qt_view(Dc, c0, c1):
                if c1 <= 512:
                    return xs[Dc][:, OF["qta"] + c0:OF["qta"] + c1]
                return xs[Dc][:, OF["qtb"] + c0 - 512:OF["qtb"] + c1 - 512]

            # gpsimd queue: side data + iota (small transfers only)
            bthr_sb = sbt("bthr_sb", [128, nband], F32)
            nc.gpsimd.dma_start(out=bthr_sb, in_=bthr[:])
            vbias_sb = sbt("vbias_sb", [128, NCH], F32)
            nc.gpsimd.dma_start(out=vbias_sb, in_=vbias[:])
            qmask_sb = sbt("qmask_sb", [128, 4 * NQ], F32)
            nc.gpsimd.dma_start(out=qmask_sb, in_=qmask[:])
            fvec_sb = sbt("fvec_sb", [128, 16], BF16)
            nc.gpsimd.dma_start(out=fvec_sb, in_=fvec[:])
            cnt_sb = sbt("cnt_sb", [128, 4], F32)
            nc.gpsimd.dma_start(out=cnt_sb, in_=cnt[:])
            ident_sb = sbt("ident_sb", [128, 128], BF16)
            nc.gpsimd.dma_start(out=ident_sb, in_=ident[:])
            iota_sb = sbt("iota_sb", [128, 512], I32)
            nc.gpsimd.iota(iota_sb, [[1, 512]], channel_multiplier=0)

            # CRITICAL DMA RULE: compute instructions on a queue wait for ALL
            # DMAs previously issued from that queue (coarse aggregated
            # semaphores).  The scalar ring carries ONLY tranche quadrants
            # 0-1 (landing before the first exp); everything else rides sync,
            # bulk split per segment in consumption order (v, k, q).  The
            # tranche is two DMAs per quadrant ([wall|k] then [v|q]) so the
            # k projection starts before the q/v columns land.
            for c0, c1 in ((0, OF["vta"]), (OF["vta"], OF["tr_end"])):
                for Dc in (0, 1):
                    nc.scalar.dma_start(out=xs[Dc][:, c0:c1],
                                        in_=xin[128 * Dc:128 * (Dc + 1), c0:c1])
                for Dc in (2, 3):
                    nc.sync.dma_start(out=xs[Dc][:, c0:c1],
                                      in_=xin[128 * Dc:128 * (Dc + 1), c0:c1])
            for seg in ("vtb", "ktb", "qtb"):
                c0, c1 = OF[seg], OF[{"vtb": "ktb", "ktb": "qtb", "qtb": "end"}[seg]]
                if c1 > c0:
                    for Dc in range(4):
                        nc.sync.dma_start(out=xs[Dc][:, c0:c1],
                                          in_=xin[128 * Dc:128 * (Dc + 1), c0:c1])

            bmask_sb = sbt("bmask_sb", [128, nband * 512], BF16)
            bdone = set()

            def ensure_bmask(t):
                for i, (c, tt) in enumerate(band_list):
                    if tt == t and i not in bdone:
                        bdone.add(i)
                        nc.vector.tensor_scalar(
                            bmask_sb[:, 512 * i:512 * (i + 1)], iota_sb,
                            bthr_sb[:, i:i + 1], None, mybir.AluOpType.is_ge)

            qwT = [sbt(f"qwT{i}", [128, NQT], BF16) for i in range(2)]
            kwT = [sbt(f"kwT{i}", [128, klim], BF16) for i in range(2)]
            vw = [sbt(f"vw{i}", [128, 4 * KS], BF16) for i in range(NU)]

            # quadrants 0/2 ride different rings and land together; (0,2,1,3)
            # starts the accumulation before quadrants 1/3 arrive
            DCO = (0, 2, 1, 3)

            def vproj(st):
                p = ps.tile([128, HG], F32, name="pprj", tag="psP", bufs=2)
                for i, Dc in enumerate(DCO):
                    nc.tensor.matmul(p, vt_view(Dc, st),
                                     w_sb[("v", Dc)], start=(i == 0), stop=(i == 3))
                t = vw[st]
                nc.vector.memset(
                    t.rearrange("p (h j) -> p h j", j=KS)[:, :, 64:65], 1.0)
                nc.vector.tensor_copy(
                    t.rearrange("p (h j) -> p h j", j=KS)[:, :, 0:64],
                    p.rearrange("p (h j) -> p h j", j=64))

            def proj_kq(dc, which, st2):
                view, dst, wnm, lim = ((kt_view, kwT, "k", klim) if which == "k"
                                       else (qt_view, qwT, "q", NQT))
                w = min(512, lim - 512 * st2)
                p = ps.tile([128, 512], F32, name="pprj2", tag="psP", bufs=2)
                for i, Dc in enumerate(DCO):
                    nc.tensor.matmul(
                        p[:, 0:w], w_sb[(wnm, Dc)][:, 128 * dc:128 * (dc + 1)],
                        view(Dc, 512 * st2, 512 * st2 + w),
                        start=(i == 0), stop=(i == 3))
                # all proj copies ride DVE: anything on the scalar queue
                # ahead of an exp delays the exp stream (in-order queues).
                # the first k tile's chunk-0 columns get their own copy so
                # the first scores don't wait for the whole 512-col cast
                if which == "k" and st2 == 0:
                    nc.any.tensor_copy(dst[dc][:, 0:128], p[:, 0:128])
                    nc.any.tensor_copy(dst[dc][:, 128:512], p[:, 128:512])
                else:
                    nc.any.tensor_copy(dst[dc][:, 512 * st2:512 * st2 + w],
                                       p[:, 0:w])

            # projection work for pass p+1 is spread through pass p's chunk
            # stream (one op per chunk) so the PE fills ACT-paced slack and
            # no projection burst starves the exp stream at pass boundaries;
            # only the minimal (k0, q0, v0) feed for pass (0,0) runs upfront
            vdone = [0]
            kdone = [0, 0]
            qdone = [0, 0]

            def proj_needs(t, dc):
                lst = []
                lc = live_lists[t]
                while vdone[0] < lc[-1] + 1:
                    st = vdone[0]
                    lst.append(lambda st=st: vproj(st))
                    vdone[0] += 1
                need_k = min(kp_tiles, -(-(128 * (lc[-1] + 1)) // 512))
                while kdone[dc] < need_k:
                    s = kdone[dc]
                    lst.append(lambda dc=dc, s=s: proj_kq(dc, "k", s))
                    kdone[dc] += 1
                while qdone[dc] < t + 1:
                    s = qdone[dc]
                    lst.append(lambda dc=dc, s=s: proj_kq(dc, "q", s))
                    qdone[dc] += 1
                return lst

            # ---- attention: q-tile passes, dc-interleaved, compacted keys ----
            # finalize of pass p is emitted after pass p+1's chunk stream so
            # the PE never stalls on the DVE psO->SBUF copy at pass ends
            ofin = sbt("ofin", [128, 4 * NQ * HG], BF16)

            def emit_ot_copies(dc, psO):
                # DVE copies psO -> SBUF right at pass end, freeing the psO
                # banks before the next pass's first PV (psO bufs=2); they
                # hide under the next pass's first exp
                ots = {}
                for hh in (2 * dc, 2 * dc + 1):
                    ot = sb.tile([KS, 512], BF16, name="ot", tag="ot", bufs=4)
                    nc.any.tensor_copy(ot, psO[hh])
                    ots[hh] = ot
                return ots

            def make_finalize(t, dc, ots):
                def fin():
                    h0, h1 = 2 * dc, 2 * dc + 1
                    for hh in (h0, h1):
                        ot = ots[hh]
                        tp = ps.tile([128, 4 * 66], BF16, name="tp", tag="psP",
                                     bufs=2)
                        for j in range(4):
                            nc.tensor.matmul(tp[:, 66 * j:66 * j + KS],
                                             ot[:, 128 * j:128 * (j + 1)],
                                             ident_sb[0:KS, 0:KS],
                                             is_transpose=True,
                                             start=(j == 0), stop=(j == 3),
                                             skip_group_check=True)
                        rs = sb.tile([128, 4], F32, name="rs", tag="rs", bufs=4)
                        if t == 0:
                            nc.vector.tensor_add(
                                rs.rearrange("p (j o) -> p j o", o=1),
                                tp.rearrange("p (j f) -> p j f", f=66)[:, :, 64:65],
                                cnt_sb.rearrange("p (j o) -> p j o", o=1))
                        else:
                            nc.vector.tensor_scalar_add(
                                rs.rearrange("p (j o) -> p j o", o=1),
                                tp.rearrange("p (j f) -> p j f", f=66)[:, :, 64:65],
                                1e-30)
                        rcp = sb.tile([128, 4], F32, name="rcp", tag="rcp", bufs=4)
                        nc.vector.reciprocal(rcp, rs)
                        scl = sb.tile([128, 4], F32, name="scl", tag="scl", bufs=4)
                        nc.vector.tensor_mul(scl, rcp, qmask_sb[:, 4 * t:4 * (t + 1)])
                        for j in range(4):
                            col = 512 * (2 * t + dc) + 128 * j + 64 * (hh - 2 * dc)
                            nc.vector.tensor_scalar_mul(
                                ofin[:, col:col + 64], tp[:, 66 * j:66 * j + 64],
                                scl[:, j:j + 1])
                    # per-pass contiguous 1KB-per-row output block; the last
                    # pass's DMA is only 0.13MB of tail
                    b = 2 * t + dc
                    nc.sync.dma_start(out=out[:, 512 * b:512 * (b + 1)],
                                      in_=ofin[:, 512 * b:512 * (b + 1)])
                return fin

            passes = [(t, dc) for t in range(NQ) for dc in range(2)]
            # minimal upfront feed for pass (0,0): k tile 0, q tile 0, v chunk 0
            proj_kq(0, "k", 0)
            kdone[0] = 1
            proj_kq(0, "q", 0)
            qdone[0] = 1
            vproj(0)
            vdone[0] = 1
            ensure_bmask(0)

            # global prework queue: (deadline pass index, fn).  Items drain
            # one-or-more per chunk across ALL passes (not just the pass
            # right before their deadline), so no projection burst piles up
            # at a tile boundary; the end-of-pass flush guarantees deadlines.
            queue = [(0, f) for f in proj_needs(0, 0)]
            for pj in range(1, len(passes)):
                tj, dcj = passes[pj]
                queue += [(pj, f) for f in proj_needs(tj, dcj)]
                if dcj == 0:
                    queue.append((pj, lambda tj=tj: ensure_bmask(tj)))

            def drain(n):
                for _ in range(n):
                    if queue:
                        queue.pop(0)[1]()

            pending = None
            for pi, (t, dc) in enumerate(passes):
                    h0, h1 = 2 * dc, 2 * dc + 1
                    kw_t, qw_t = kwT[dc], qwT[dc]
                    lc = live_lists[t]
                    fin_at = min(2, len(lc) - 1)
                    psO = {}
                    for hh in (h0, h1):
                        psO[hh] = ps.tile([KS, 512], F32, name=f"psO{hh}",
                                          tag="psO", bufs=2)
                    def emit_pv(c, U):
                        # band-mask multiply + PV for both heads of chunk c
                        o = qlo.get((c, t), 0)
                        for i, hh in enumerate((h0, h1)):
                            Ui = U[:, 512 * i + o:512 * (i + 1)]
                            if (c, t) in band_idx:
                                off = band_idx[(c, t)] * 512
                                nc.any.tensor_mul(
                                    Ui, Ui, bmask_sb[:, off + o:off + 512])
                            stop = (c == lc[-1]) if t > 0 else False
                            nc.tensor.matmul(psO[hh][:, o:],
                                             vw[c][:, KS * hh:KS * (hh + 1)],
                                             Ui,
                                             start=(c == lc[0]), stop=stop,
                                             skip_group_check=True)

                    # software pipeline: scores/exp of chunk c are emitted
                    # before PV of chunk c-1, so the in-order PE never waits
                    # on the exp -> mask -> PV chain of the previous chunk
                    ci = 0
                    prev_pv = None
                    for c in range(lc[-1] + 1):
                        if c in lc:
                            o = qlo.get((c, t), 0)
                            psS = ps.tile([128, 1024], F32, name="psS",
                                          tag="psS", bufs=2)
                            for i, ho in enumerate((0, 64)):
                                nc.tensor.matmul(
                                    psS[:, 512 * i + o:512 * (i + 1)],
                                    kw_t[ho:ho + 64, 128 * c:128 * (c + 1)],
                                    qw_t[ho:ho + 64, 512 * t + o:512 * (t + 1)],
                                    start=True, stop=True)
                            U = sb.tile([128, 1024], BF16, name="U", tag="U",
                                        bufs=8)
                            nc.scalar.activation(
                                U.rearrange("p (i q) -> p i q", q=512)[:, :, o:],
                                psS.rearrange("p (i q) -> p i q", q=512)[:, :, o:],
                                Exp, bias=vbias_sb[:, c:c + 1], scale=0.125)
                            if prev_pv is not None:
                                emit_pv(*prev_pv)
                            prev_pv = (c, U)
                            if ci == fin_at and pending is not None:
                                pending()
                                pending = None
                            else:
                                # pace the queue so everything due by the
                                # next pass start is emitted in time
                                due = sum(1 for d, _ in queue if d <= pi + 1)
                                slots = max(1, len(lc) - 1 - ci)
                                drain(max(1 if queue else 0,
                                          -(-due // slots)))
                            ci += 1
                    emit_pv(*prev_pv)
                    while queue and queue[0][0] <= pi + 1:
                        queue.pop(0)[1]()
                    if t == 0:
                        # dead-query fix: psO[:, 0:4] += Wv_hh^T @ fvec
                        for hh in (h0, h1):
                            for Dc in range(4):
                                nc.tensor.matmul(
                                    psO[hh][0:64, 0:4],
                                    w_sb[("v", Dc)][:, 64 * hh:64 * (hh + 1)],
                                    fvec_sb[:, 4 * Dc:4 * (Dc + 1)],
                                    start=False, stop=(Dc == 3),
                                    skip_group_check=True)
                    ots = emit_ot_copies(dc, psO)
                    pending = make_finalize(t, dc, ots)
            pending()

    nc.compile()
    return nc


def _prep_inputs(q, k, v, v_mask, q_mask, Wq, Wk, Wv,
                 perms, n1s, qposs, NU, NQ, band_list):
    q = np.asarray(q, np.float32)
    k = np.asarray(k, np.float32)
    v = np.asarray(v, np.float32)
    v_mask = np.asarray(v_mask, np.float32)
    Wq = np.asarray(Wq, np.float32)
    Wk = np.asarray(Wk, np.float32)
    Wv = np.asarray(Wv, np.float32)
    ident = np.eye(128, dtype=np.float32)
    nband = len(band_list)
    klim = NU * 128
    NQT = NQ * 512
    OF = _offsets(NU, NQ)

    in_maps = []
    for core in range(8):
        b, hg = core // 2, core % 2
        cs = slice(hg * HG, (hg + 1) * HG)
        perm, n1, qp = perms[b], n1s[b], qposs[b]
        nq = len(qp)
        # compacted query order, padded with row 0 (masked off via qmask)
        qperm = np.concatenate([qp, np.zeros(NQT - nq, np.int64)])
        kTc = k[b][perm].T.astype(BF)     # [D, S]
        vTc = v[b][perm].T.astype(BF)
        qTc = q[b][qperm].T.astype(BF)    # [D, NQT]
        wallc = np.concatenate([Wq[:, cs], Wk[:, cs], Wv[:, cs]],
                               axis=1).astype(BF)  # [D, 768]
        xin = np.empty((D, OF["end"]), BF)
        xin[:, OF["wall"]:OF["wall"] + 768] = wallc
        xin[:, OF["kta"]:OF["kta"] + 512] = kTc[:, 0:512]
        xin[:, OF["vta"]:OF["vta"] + 128] = vTc[:, 0:128]
        xin[:, OF["qta"]:OF["qta"] + 512] = qTc[:, 0:512]
        xin[:, OF["vtb"]:OF["vtb"] + klim - 128] = vTc[:, 128:klim]
        xin[:, OF["ktb"]:OF["ktb"] + klim - 512] = kTc[:, 512:klim]
        if NQT > 512:
            xin[:, OF["qtb"]:OF["qtb"] + NQT - 512] = qTc[:, 512:NQT]
        vb = np.where(np.arange(S) < n1, np.float32(0), NEG).astype(np.float32)
        qm_c = np.zeros(NQT, np.float32)
        qm_c[:nq] = 1.0
        # dead queries: pos < first unmasked key, q_mask==1; they compact
        # to the first columns of tile 0
        fix = np.zeros((S, 4), np.float32)
        cnt = np.full((128, 4), np.float32(1e-30))
        if v_mask[b, 0] == 0:
            first_one = int(np.argmax(v_mask[b] > 0))
            ks_ = np.arange(S)
            nd = int((qp < first_one).sum())
            for dj in range(min(nd, 4)):
                pj = int(qp[dj])
                sel = ((ks_ <= pj) & (v_mask[b] == 0)) | \
                      ((ks_ > pj) & (v_mask[b] == 1))
                fix[:, dj] = sel[perm].astype(np.float32)
        fvec = (v[b][perm].T @ fix).astype(np.float32)
        cnt[0:4, 0] += fix.sum(axis=0)
        # per-band threshold in compacted index space:
        # mask[k, qc] = (qc >= searchsorted(qp, pos_k) - 512 t)
        bthr = np.zeros((128, nband), np.float32)
        for i, (c, t) in enumerate(band_list):
            kpos = perm[128 * c:128 * (c + 1)]
            bthr[:, i] = np.searchsorted(qp, kpos).astype(np.float32) - 512.0 * t
        in_maps.append({
            "xin": np.ascontiguousarray(xin),
            "vbias": np.ascontiguousarray(vb.reshape(NCH, 128).T),
            "qmask": np.ascontiguousarray(qm_c.reshape(4 * NQ, 128).T),
            "bthr": bthr,
            "fvec": np.ascontiguousarray(
                fvec.reshape(4, 128, 4).transpose(1, 0, 2)
                .reshape(128, 16)).astype(BF),
            "cnt": cnt,
            "ident": ident.astype(BF),
        })
    return in_maps


def kernel(q, k, v, v_mask, q_mask, Wq, Wk, Wv, _trace=False):
    from concourse.bass_utils import run_bass_kernel_spmd

    v_mask_f = np.asarray(v_mask, np.float32)
    q_mask_f = np.asarray(q_mask, np.float32)
    perms, n1s, qposs, NU, NQ, live_lists, band_list, qlo_t = \
        _structure(v_mask_f, q_mask_f)
    key = (NU, NQ, live_lists, band_list, qlo_t)
    if _CACHE.get("key") != key:
        _CACHE["nc"] = _build(NU, NQ, live_lists, band_list, qlo_t)
        _CACHE["key"] = key
    nc = _CACHE["nc"]
    in_maps = _prep_inputs(q, k, v, v_mask, q_mask, Wq, Wk, Wv,
                           perms, n1s, qposs, NU, NQ, band_list)
    res = run_bass_kernel_spmd(nc, in_maps, core_ids=list(range(8)), trace=_trace)
    _CACHE["last_result"] = res
    NQT = NQ * 512
    full = np.zeros((B, S, 2 * HG), np.float32)
    for core in range(8):
        b, hg = core // 2, core % 2
        o = np.asarray(res.results[core]["out"], np.float32)
        # [128, 2NQ*512] blocks b=2t+dc of [j(4), n(128)] ->
        # row 512t+128j+p, cols 128dc+n
        o = o.reshape(128, NQ, 2, 4, 128).transpose(1, 3, 0, 2, 4) \
             .reshape(NQT, 2 * 128)
        full[b, qposs[b], hg * HG:(hg + 1) * HG] = o[:len(qposs[b])]
    return full


# revision 72
# speedup vs baseline: 1.0051x; 1.0051x over previous
"""Distributed multi-head attention kernel for 8 TRN2 NeuronCores.

Sharding: core c handles batch b = c//2 and head-group hg = c%2 (4 of 8
heads = 256 output columns).  Output slices are disjoint -> no collectives;
the host concatenates the 8 slices (bf16 device output, f32 host).

Device algorithm (per core), bf16 matmuls / f32 softmax:
  - host compacts BOTH axes: keys permuted unmasked-first (sparse
    attention over v_mask) and queries compacted to q_mask==1 only
    (dead queries are exactly zero in the reference); only NU=
    ceil(max_unmasked/128) key chunks and NQ=ceil(max_live_q/512)
    query tiles enter the pipeline.  Causal-mask thresholds move to
    compacted index space via host-side searchsorted, so the device
    masking (iota + per-partition is_ge) is unchanged.
  - DMA is descriptor-rate bound (~61ns per <=1KB row on each of 16
    SDMA engines), so inputs are packed per 128-partition quadrant
    into one [128, W] dram tensor with ~4KB rows: a "tranche" segment
    (wall | k 0:512 | v 0:128 | q 0:512) that pass (0,0) consumes,
    split [wall|k]/[v|q] so the k projection starts early, then bulk
    segments in consumption order (v | k | q) -- finer splits lose:
    each DMA issue costs ~690ns of sequencer time.  The scalar ring
    carries only tranche quadrants 0-1 (its compute instructions
    inherit waits on DMAs issued from the same queue, and the first
    exp must not wait on bulk); sync carries the rest; side data +
    iota ride gpsimd-SWDGE (small only: tiny-row descriptors on the
    sync ring would delay the tranche).  Output is written per pass
    as a contiguous [128, 512] block (1KB rows) of the persistent
    ofin tile; the host re-interleaves blocks b=2t+dc.
  - scores in S^T layout [k', q]; the two heads of a pair use PE row
    groups 0-63 / 64-127 so their score matmuls run concurrently; one
    [128, 1024] PSUM tile holds both heads' scores for a q-tile and a
    single ACT exp (per-partition key bias; scale=0.125) covers both
  - causal masking: block-level skips from a union-over-batches
    liveness structure (SPMD-identical graph); straddling blocks are
    trimmed to q >= qlo (union) in scores/exp/mask/PV, and the
    residual staircase masks are generated on device (gpsimd iota +
    vector is_ge against per-band thresholds)
  - PV: O^T[65, q] accumulated in PSUM over key chunks; row 64 (ones
    column appended to VW) is the softmax denominator
  - the (q-tile t, head-pair dc) passes are emitted t-major; all
    projections go into a global deadline queue drained one-or-more
    items per chunk across every pass, bounding projection bursts at
    tile boundaries; scores are emitted one chunk ahead of the
    previous chunk's PV (psS bufs=2 is scores-only, proj psum has its
    own 2-buf pool) so the in-order PE never waits on the
    exp -> mask -> PV chain; ALL psum->SBUF copies ride DVE because
    anything queued ahead of an exp on the scalar queue delays the
    exp stream; the psO->SBUF copies run at pass end (psO bufs=2,
    hidden under the next pass's first exp) while transposes + output
    scaling are deferred to the next pass's third chunk
  - dead queries (all causally-allowed keys masked but q_mask==1):
    host precomputes fvec = v_perm^T @ F at their compacted indices
    (they compact to the first columns); 16 tiny matmuls add the fix
    into output columns 0..3, with dead-slot counts joined at finalize
  - finalize per pass: DVE-copy psO to SBUF (bf16), PE-transpose to
    [q, 65], scale by q_mask/rowsum into ofin; host scatters the
    compacted rows back to full [S, 512]
"""

import numpy as np
import ml_dtypes

BF = ml_dtypes.bfloat16
B, S, D = 4, 2048, 512
HG = 256          # output columns per core (4 heads x 64)
KS = 65           # head value width + ones column
NCH = 16          # total key chunks of 128
NEG = np.float32(-1e10)

_CACHE = {}


def _structure(v_mask, q_mask):
    """Key/query compaction + block liveness (union over batches)."""
    perms, n1s, qposs = [], [], []
    for b in range(B):
        unm = np.where(v_mask[b] == 1)[0]
        msk = np.where(v_mask[b] == 0)[0]
        perms.append(np.concatenate([unm, msk]))
        n1s.append(len(unm))
        qposs.append(np.where(q_mask[b] == 1)[0])
    NU = int(max(-(-n // 128) for n in n1s))
    NQ = int(max(-(-len(qp) // 512) for qp in qposs))
    live = set()
    band = set()
    qlo_raw = {}
    for b in range(B):
        unm = perms[b][:n1s[b]]
        qp = qposs[b]
        nq = len(qp)
        for c in range(NU):
            seg = unm[128 * c:min(128 * (c + 1), n1s[b])]
            if len(seg) == 0:
                continue
            lo, hi = int(seg[0]), int(seg[-1])
            # compacted index of the first query that sees lo / all of hi
            qlo_c = int(np.searchsorted(qp, lo))
            qhi_c = int(np.searchsorted(qp, hi))
            for t in range(NQ):
                tile_last = min(512 * (t + 1), nq) - 1
                if tile_last < 512 * t or qlo_c > tile_last:
                    continue
                live.add((c, t))
                ql = max(0, qlo_c - 512 * t)
                qlo_raw[(c, t)] = min(qlo_raw.get((c, t), 512), ql)
                if qhi_c > 512 * t:
                    band.add((c, t))
    live_lists = tuple(tuple(sorted(c for (c, tt) in live if tt == t))
                       for t in range(NQ))
    band_list = tuple(sorted(band))
    # queries below qlo see no key of the chunk (union over batches); the
    # first live chunk of each tile keeps full width (starts the psO group)
    qlo = {}
    for (c, t), v in qlo_raw.items():
        qlo[(c, t)] = 0 if c == live_lists[t][0] else (v // 8) * 8
    qlo_t = tuple(sorted(qlo.items()))
    return perms, n1s, qposs, NU, NQ, live_lists, band_list, qlo_t


def _offsets(NU, NQ):
    """Column offsets inside the packed per-quadrant input tensor."""
    klim, NQT = NU * 128, NQ * 512
    o = {}
    o["wall"] = 0
    o["kta"] = 768
    o["vta"] = 768 + 512
    o["qta"] = o["vta"] + 128
    o["tr_end"] = o["qta"] + 512          # tranche = first-pass feed
    o["vtb"] = o["tr_end"]
    o["ktb"] = o["vtb"] + (klim - 128)
    o["qtb"] = o["ktb"] + (klim - 512)
    o["end"] = o["qtb"] + (NQT - 512)
    return o


def _build(NU, NQ, live_lists, band_list, qlo_t):
    import concourse.bass as bass  # noqa: F401
    from concourse import bacc
    import concourse.mybir as mybir
    from concourse.tile import TileContext

    F32 = mybir.dt.float32
    BF16 = mybir.dt.bfloat16
    I32 = mybir.dt.int32
    Exp = mybir.ActivationFunctionType.Exp
    nband = len(band_list)
    band_idx = {ct: i for i, ct in enumerate(band_list)}
    qlo = dict(qlo_t)
    klim = NU * 128
    NQT = NQ * 512
    kp_tiles = -(-klim // 512)  # s-tiles of K to project
    OF = _offsets(NU, NQ)

    nc = bacc.Bacc()
    xin = nc.declare_dram_parameter("xin", [D, OF["end"]], BF16, isOutput=False)
    vbias = nc.declare_dram_parameter("vbias", [128, NCH], F32, isOutput=False)
    qmask = nc.declare_dram_parameter("qmask", [128, 4 * NQ], F32, isOutput=False)
    bthr = nc.declare_dram_parameter("bthr", [128, nband], F32, isOutput=False)
    fvec = nc.declare_dram_parameter("fvec", [128, 16], BF16, isOutput=False)
    cnt = nc.declare_dram_parameter("cnt", [128, 4], F32, isOutput=False)
    ident = nc.declare_dram_parameter("ident", [128, 128], BF16, isOutput=False)
    out = nc.declare_dram_parameter("out", [128, 4 * NQ * HG], BF16, isOutput=True)

    with TileContext(nc) as tc:
        with tc.tile_pool(name="sb", bufs=1) as sb, \
             tc.tile_pool(name="ps", bufs=1, space="PSUM") as ps:

            def sbt(name, shape, dtype, bufs=1, tag=None):
                return sb.tile(shape, dtype, name=name, tag=tag or name, bufs=bufs)

            xs = [sb.tile([128, OF["end"]], BF16, name=f"xin{Dc}",
                          tag=f"xin{Dc}", bufs=1) for Dc in range(4)]

            w_sb = {}
            for Dc in range(4):
                for j, nm in enumerate(("q", "k", "v")):
                    w_sb[(nm, Dc)] = xs[Dc][:, HG * j:HG * (j + 1)]

            def kt_view(Dc, c0, c1):
                if c1 <= 512:
                    return xs[Dc][:, OF["kta"] + c0:OF["kta"] + c1]
                return xs[Dc][:, OF["ktb"] + c0 - 512:OF["ktb"] + c1 - 512]

            def vt_view(Dc, st):
                if st == 0:
                    return xs[Dc][:, OF["vta"]:OF["vta"] + 128]
                return xs[Dc][:, OF["vtb"] + 128 * (st - 1):OF["vtb"] + 128 * st]

            def qt_view(Dc, c0, c1):
                if c1 <= 512:
                    return xs[Dc][:, OF["qta"] + c0:OF["qta"] + c1]
                return xs[Dc][:, OF["qtb"] + c0 - 512:OF["qtb"] + c1 - 512]

            # gpsimd queue: side data + iota (small transfers only)
            bthr_sb = sbt("bthr_sb", [128, nband], F32)
            nc.gpsimd.dma_start(out=bthr_sb, in_=bthr[:])
            vbias_sb = sbt("vbias_sb", [128, NCH], F32)
            nc.gpsimd.dma_start(out=vbias_sb, in_=vbias[:])
            qmask_sb = sbt("qmask_sb", [128, 4 * NQ], F32)
            nc.gpsimd.dma_start(out=qmask_sb, in_=qmask[:])
            fvec_sb = sbt("fvec_sb", [128, 16], BF16)
            nc.gpsimd.dma_start(out=fvec_sb, in_=fvec[:])
            cnt_sb = sbt("cnt_sb", [128, 4], F32)
            nc.gpsimd.dma_start(out=cnt_sb, in_=cnt[:])
            ident_sb = sbt("ident_sb", [128, 128], BF16)
            nc.gpsimd.dma_start(out=ident_sb, in_=ident[:])
            iota_sb = sbt("iota_sb", [128, 512], I32)
            nc.gpsimd.iota(iota_sb, [[1, 512]], channel_multiplier=0)

            # CRITICAL DMA RULE: compute instructions on a queue wait for ALL
            # DMAs previously issued from that queue (coarse aggregated
            # semaphores).  The scalar ring carries ONLY tranche quadrants
            # 0-1 (landing before the first exp); everything else rides sync,
            # bulk split per segment in consumption order (v, k, q).  The
            # tranche is two DMAs per quadrant ([wall|k] then [v|q]) so the
            # k projection starts before the q/v columns land.
            for c0, c1 in ((0, OF["vta"]), (OF["vta"], OF["tr_end"])):
                for Dc in (0, 1):
                    nc.scalar.dma_start(out=xs[Dc][:, c0:c1],
                                        in_=xin[128 * Dc:128 * (Dc + 1), c0:c1])
                for Dc in (2, 3):
                    nc.sync.dma_start(out=xs[Dc][:, c0:c1],
                                      in_=xin[128 * Dc:128 * (Dc + 1), c0:c1])
            for seg in ("vtb", "ktb", "qtb"):
                c0, c1 = OF[seg], OF[{"vtb": "ktb", "ktb": "qtb", "qtb": "end"}[seg]]
                if c1 > c0:
                    for Dc in range(4):
                        nc.sync.dma_start(out=xs[Dc][:, c0:c1],
                                          in_=xin[128 * Dc:128 * (Dc + 1), c0:c1])

            bmask_sb = sbt("bmask_sb", [128, nband * 512], BF16)
            bdone = set()

            def ensure_bmask(t):
                for i, (c, tt) in enumerate(band_list):
                    if tt == t and i not in bdone:
                        bdone.add(i)
                        nc.vector.tensor_scalar(
                            bmask_sb[:, 512 * i:512 * (i + 1)], iota_sb,
                            bthr_sb[:, i:i + 1], None, mybir.AluOpType.is_ge)

            qwT = [sbt(f"qwT{i}", [128, NQT], BF16) for i in range(2)]
            kwT = [sbt(f"kwT{i}", [128, klim], BF16) for i in range(2)]
            vw = [sbt(f"vw{i}", [128, 4 * KS], BF16) for i in range(NU)]

            # quadrants 0/2 ride different rings and land together; (0,2,1,3)
            # starts the accumulation before quadrants 1/3 arrive
            DCO = (0, 2, 1, 3)

            def vproj(st):
                p = ps.tile([128, HG], F32, name="pprj", tag="psP", bufs=2)
                for i, Dc in enumerate(DCO):
                    nc.tensor.matmul(p, vt_view(Dc, st),
                                     w_sb[("v", Dc)], start=(i == 0), stop=(i == 3))
                t = vw[st]
                nc.vector.memset(
                    t.rearrange("p (h j) -> p h j", j=KS)[:, :, 64:65], 1.0)
                nc.vector.tensor_copy(
                    t.rearrange("p (h j) -> p h j", j=KS)[:, :, 0:64],
                    p.rearrange("p (h j) -> p h j", j=64))

            def proj_kq(dc, which, st2):
                view, dst, wnm, lim = ((kt_view, kwT, "k", klim) if which == "k"
                                       else (qt_view, qwT, "q", NQT))
                w = min(512, lim - 512 * st2)
                p = ps.tile([128, 512], F32, name="pprj2", tag="psP", bufs=2)
                for i, Dc in enumerate(DCO):
                    nc.tensor.matmul(
                        p[:, 0:w], w_sb[(wnm, Dc)][:, 128 * dc:128 * (dc + 1)],
                        view(Dc, 512 * st2, 512 * st2 + w),
                        start=(i == 0), stop=(i == 3))
                # all proj copies ride DVE: anything on the scalar queue
                # ahead of an exp delays the exp stream (in-order queues).
                # the first k tile's chunk-0 columns get their own copy so
                # the first scores don't wait for the whole 512-col cast
                if which == "k" and st2 == 0:
                    nc.vector.tensor_copy(dst[dc][:, 0:128], p[:, 0:128])
                    nc.vector.tensor_copy(dst[dc][:, 128:512], p[:, 128:512])
                else:
                    nc.vector.tensor_copy(dst[dc][:, 512 * st2:512 * st2 + w],
                                          p[:, 0:w])

            # projection work for pass p+1 is spread through pass p's chunk
            # stream (one op per chunk) so the PE fills ACT-paced slack and
            # no projection burst starves the exp stream at pass boundaries;
            # only the minimal (k0, q0, v0) feed for pass (0,0) runs upfront
            vdone = [0]
            kdone = [0, 0]
            qdone = [0, 0]

            def proj_needs(t, dc):
                lst = []
                lc = live_lists[t]
                while vdone[0] < lc[-1] + 1:
                    st = vdone[0]
                    lst.append(lambda st=st: vproj(st))
                    vdone[0] += 1
                need_k = min(kp_tiles, -(-(128 * (lc[-1] + 1)) // 512))
                while kdone[dc] < need_k:
                    s = kdone[dc]
                    lst.append(lambda dc=dc, s=s: proj_kq(dc, "k", s))
                    kdone[dc] += 1
                while qdone[dc] < t + 1:
                    s = qdone[dc]
                    lst.append(lambda dc=dc, s=s: proj_kq(dc, "q", s))
                    qdone[dc] += 1
                return lst

            # ---- attention: q-tile passes, dc-interleaved, compacted keys ----
            # finalize of pass p is emitted after pass p+1's chunk stream so
            # the PE never stalls on the DVE psO->SBUF copy at pass ends
            ofin = sbt("ofin", [128, 4 * NQ * HG], BF16)

            def emit_ot_copies(dc, psO):
                # DVE copies psO -> SBUF right at pass end, freeing the psO
                # banks before the next pass's first PV (psO bufs=2); they
                # hide under the next pass's first exp
                ots = {}
                for hh in (2 * dc, 2 * dc + 1):
                    ot = sb.tile([KS, 512], BF16, name="ot", tag="ot", bufs=4)
                    nc.vector.tensor_copy(ot, psO[hh])
                    ots[hh] = ot
                return ots

            def make_finalize(t, dc, ots):
                def fin():
                    h0, h1 = 2 * dc, 2 * dc + 1
                    for hh in (h0, h1):
                        ot = ots[hh]
                        tp = ps.tile([128, 4 * 66], BF16, name="tp", tag="psP",
                                     bufs=2)
                        for j in range(4):
                            nc.tensor.matmul(tp[:, 66 * j:66 * j + KS],
                                             ot[:, 128 * j:128 * (j + 1)],
                                             ident_sb[0:KS, 0:KS],
                                             is_transpose=True,
                                             start=(j == 0), stop=(j == 3),
                                             skip_group_check=True)
                        rs = sb.tile([128, 4], F32, name="rs", tag="rs", bufs=4)
                        if t == 0:
                            nc.vector.tensor_add(
                                rs.rearrange("p (j o) -> p j o", o=1),
                                tp.rearrange("p (j f) -> p j f", f=66)[:, :, 64:65],
                                cnt_sb.rearrange("p (j o) -> p j o", o=1))
                        else:
                            nc.vector.tensor_scalar_add(
                                rs.rearrange("p (j o) -> p j o", o=1),
                                tp.rearrange("p (j f) -> p j f", f=66)[:, :, 64:65],
                                1e-30)
                        rcp = sb.tile([128, 4], F32, name="rcp", tag="rcp", bufs=4)
                        nc.vector.reciprocal(rcp, rs)
                        scl = sb.tile([128, 4], F32, name="scl", tag="scl", bufs=4)
                        nc.vector.tensor_mul(scl, rcp, qmask_sb[:, 4 * t:4 * (t + 1)])
                        for j in range(4):
                            col = 512 * (2 * t + dc) + 128 * j + 64 * (hh - 2 * dc)
                            nc.vector.tensor_scalar_mul(
                                ofin[:, col:col + 64], tp[:, 66 * j:66 * j + 64],
                                scl[:, j:j + 1])
                    # per-pass contiguous 1KB-per-row output block; the last
                    # pass's DMA is only 0.13MB of tail
                    b = 2 * t + dc
                    nc.sync.dma_start(out=out[:, 512 * b:512 * (b + 1)],
                                      in_=ofin[:, 512 * b:512 * (b + 1)])
                return fin

            passes = [(t, dc) for t in range(NQ) for dc in range(2)]
            # minimal upfront feed for pass (0,0): k tile 0, q tile 0, v chunk 0
            proj_kq(0, "k", 0)
            kdone[0] = 1
            proj_kq(0, "q", 0)
            qdone[0] = 1
            vproj(0)
            vdone[0] = 1
            ensure_bmask(0)

            # global prework queue: (deadline pass index, fn).  Items drain
            # one-or-more per chunk across ALL passes (not just the pass
            # right before their deadline), so no projection burst piles up
            # at a tile boundary; the end-of-pass flush guarantees deadlines.
            queue = [(0, f) for f in proj_needs(0, 0)]
            for pj in range(1, len(passes)):
                tj, dcj = passes[pj]
                queue += [(pj, f) for f in proj_needs(tj, dcj)]
                if dcj == 0:
                    queue.append((pj, lambda tj=tj: ensure_bmask(tj)))

            def drain(n):
                for _ in range(n):
                    if queue:
                        queue.pop(0)[1]()

            pending = None
            for pi, (t, dc) in enumerate(passes):
                    h0, h1 = 2 * dc, 2 * dc + 1
                    kw_t, qw_t = kwT[dc], qwT[dc]
                    lc = live_lists[t]
                    fin_at = min(2, len(lc) - 1)
                    psO = {}
                    for hh in (h0, h1):
                        psO[hh] = ps.tile([KS, 512], F32, name=f"psO{hh}",
                                          tag="psO", bufs=2)
                    def emit_pv(c, U):
                        # band-mask multiply + PV for both heads of chunk c
                        o = qlo.get((c, t), 0)
                        for i, hh in enumerate((h0, h1)):
                            Ui = U[:, 512 * i + o:512 * (i + 1)]
                            if (c, t) in band_idx:
                                off = band_idx[(c, t)] * 512
                                nc.vector.tensor_mul(
                                    Ui, Ui, bmask_sb[:, off + o:off + 512])
                            stop = (c == lc[-1]) if t > 0 else False
                            nc.tensor.matmul(psO[hh][:, o:],
                                             vw[c][:, KS * hh:KS * (hh + 1)],
                                             Ui,
                                             start=(c == lc[0]), stop=stop,
                                             skip_group_check=True)

                    # software pipeline: scores/exp of chunk c are emitted
                    # before PV of chunk c-1, so the in-order PE never waits
                    # on the exp -> mask -> PV chain of the previous chunk
                    ci = 0
                    prev_pv = None
                    for c in range(lc[-1] + 1):
                        if c in lc:
                            o = qlo.get((c, t), 0)
                            psS = ps.tile([128, 1024], F32, name="psS",
                                          tag="psS", bufs=2)
                            for i, ho in enumerate((0, 64)):
                                nc.tensor.matmul(
                                    psS[:, 512 * i + o:512 * (i + 1)],
                                    kw_t[ho:ho + 64, 128 * c:128 * (c + 1)],
                                    qw_t[ho:ho + 64, 512 * t + o:512 * (t + 1)],
                                    start=True, stop=True)
                            U = sb.tile([128, 1024], BF16, name="U", tag="U",
                                        bufs=8)
                            nc.scalar.activation(
                                U.rearrange("p (i q) -> p i q", q=512)[:, :, o:],
                                psS.rearrange("p (i q) -> p i q", q=512)[:, :, o:],
                                Exp, bias=vbias_sb[:, c:c + 1], scale=0.125)
                            if prev_pv is not None:
                                emit_pv(*prev_pv)
                            prev_pv = (c, U)
                            if ci == fin_at and pending is not None:
                                pending()
                                pending = None
                            else:
                                # pace the queue so everything due by the
                                # next pass start is emitted in time
                                due = sum(1 for d, _ in queue if d <= pi + 1)
                                slots = max(1, len(lc) - 1 - ci)
                                drain(max(1 if queue else 0,
                                          -(-due // slots)))
                            ci += 1
                    emit_pv(*prev_pv)
                    while queue and queue[0][0] <= pi + 1:
                        queue.pop(0)[1]()
                    if t == 0:
                        # dead-query fix: psO[:, 0:4] += Wv_hh^T @ fvec
                        for hh in (h0, h1):
                            for Dc in range(4):
                                nc.tensor.matmul(
                                    psO[hh][0:64, 0:4],
                                    w_sb[("v", Dc)][:, 64 * hh:64 * (hh + 1)],
                                    fvec_sb[:, 4 * Dc:4 * (Dc + 1)],
                                    start=False, stop=(Dc == 3),
                                    skip_group_check=True)
                    ots = emit_ot_copies(dc, psO)
                    pending = make_finalize(t, dc, ots)
            pending()

    nc.compile()
    return nc


def _prep_inputs(q, k, v, v_mask, q_mask, Wq, Wk, Wv,
                 perms, n1s, qposs, NU, NQ, band_list):
    q = np.asarray(q, np.float32)
    k = np.asarray(k, np.float32)
    v = np.asarray(v, np.float32)
    v_mask = np.asarray(v_mask, np.float32)
    Wq = np.asarray(Wq, np.float32)
    Wk = np.asarray(Wk, np.float32)
    Wv = np.asarray(Wv, np.float32)
    ident = np.eye(128, dtype=np.float32)
    nband = len(band_list)
    klim = NU * 128
    NQT = NQ * 512
    OF = _offsets(NU, NQ)

    in_maps = []
    for core in range(8):
        b, hg = core // 2, core % 2
        cs = slice(hg * HG, (hg + 1) * HG)
        perm, n1, qp = perms[b], n1s[b], qposs[b]
        nq = len(qp)
        # compacted query order, padded with row 0 (masked off via qmask)
        qperm = np.concatenate([qp, np.zeros(NQT - nq, np.int64)])
        kTc = k[b][perm].T.astype(BF)     # [D, S]
        vTc = v[b][perm].T.astype(BF)
        qTc = q[b][qperm].T.astype(BF)    # [D, NQT]
        wallc = np.concatenate([Wq[:, cs], Wk[:, cs], Wv[:, cs]],
                               axis=1).astype(BF)  # [D, 768]
        xin = np.empty((D, OF["end"]), BF)
        xin[:, OF["wall"]:OF["wall"] + 768] = wallc
        xin[:, OF["kta"]:OF["kta"] + 512] = kTc[:, 0:512]
        xin[:, OF["vta"]:OF["vta"] + 128] = vTc[:, 0:128]
        xin[:, OF["qta"]:OF["qta"] + 512] = qTc[:, 0:512]
        xin[:, OF["vtb"]:OF["vtb"] + klim - 128] = vTc[:, 128:klim]
        xin[:, OF["ktb"]:OF["ktb"] + klim - 512] = kTc[:, 512:klim]
        if NQT > 512:
            xin[:, OF["qtb"]:OF["qtb"] + NQT - 512] = qTc[:, 512:NQT]
        vb = np.where(np.arange(S) < n1, np.float32(0), NEG).astype(np.float32)
        qm_c = np.zeros(NQT, np.float32)
        qm_c[:nq] = 1.0
        # dead queries: pos < first unmasked key, q_mask==1; they compact
        # to the first columns of tile 0
        fix = np.zeros((S, 4), np.float32)
        cnt = np.full((128, 4), np.float32(1e-30))
        if v_mask[b, 0] == 0:
            first_one = int(np.argmax(v_mask[b] > 0))
            ks_ = np.arange(S)
            nd = int((qp < first_one).sum())
            for dj in range(min(nd, 4)):
                pj = int(qp[dj])
                sel = ((ks_ <= pj) & (v_mask[b] == 0)) | \
                      ((ks_ > pj) & (v_mask[b] == 1))
                fix[:, dj] = sel[perm].astype(np.float32)
        fvec = (v[b][perm].T @ fix).astype(np.float32)
        cnt[0:4, 0] += fix.sum(axis=0)
        # per-band threshold in compacted index space:
        # mask[k, qc] = (qc >= searchsorted(qp, pos_k) - 512 t)
        bthr = np.zeros((128, nband), np.float32)
        for i, (c, t) in enumerate(band_list):
            kpos = perm[128 * c:128 * (c + 1)]
            bthr[:, i] = np.searchsorted(qp, kpos).astype(np.float32) - 512.0 * t
        in_maps.append({
            "xin": np.ascontiguousarray(xin),
            "vbias": np.ascontiguousarray(vb.reshape(NCH, 128).T),
            "qmask": np.ascontiguousarray(qm_c.reshape(4 * NQ, 128).T),
            "bthr": bthr,
            "fvec": np.ascontiguousarray(
                fvec.reshape(4, 128, 4).transpose(1, 0, 2)
                .reshape(128, 16)).astype(BF),
            "cnt": cnt,
            "ident": ident.astype(BF),
        })
    return in_maps


def kernel(q, k, v, v_mask, q_mask, Wq, Wk, Wv, _trace=False):
    from concourse.bass_utils import run_bass_kernel_spmd

    v_mask_f = np.asarray(v_mask, np.float32)
    q_mask_f = np.asarray(q_mask, np.float32)
    perms, n1s, qposs, NU, NQ, live_lists, band_list, qlo_t = \
        _structure(v_mask_f, q_mask_f)
    key = (NU, NQ, live_lists, band_list, qlo_t)
    if _CACHE.get("key") != key:
        _CACHE["nc"] = _build(NU, NQ, live_lists, band_list, qlo_t)
        _CACHE["key"] = key
    nc = _CACHE["nc"]
    in_maps = _prep_inputs(q, k, v, v_mask, q_mask, Wq, Wk, Wv,
                           perms, n1s, qposs, NU, NQ, band_list)
    res = run_bass_kernel_spmd(nc, in_maps, core_ids=list(range(8)), trace=_trace)
    _CACHE["last_result"] = res
    NQT = NQ * 512
    full = np.zeros((B, S, 2 * HG), np.float32)
    for core in range(8):
        b, hg = core // 2, core % 2
        o = np.asarray(res.results[core]["out"], np.float32)
        # [128, 2NQ*512] blocks b=2t+dc of [j(4), n(128)] ->
        # row 512t+128j+p, cols 128dc+n
        o = o.reshape(128, NQ, 2, 4, 128).transpose(1, 3, 0, 2, 4) \
             .reshape(NQT, 2 * 128)
        full[b, qposs[b], hg * HG:(hg + 1) * HG] = o[:len(qposs[b])]
    return full
